# revision 1
# baseline (speedup 1.0000x reference)
"""Edge-parallel COO SpMM (segment_sum of vals * seq[cols] by sorted rows) on 8 trn2 cores.

out[r] = sum_{e: rows[e]==r} vals[e] * seq[0, cols[e], :]

rows are sorted, so shard edges by contiguous 64-row windows; core k owns
windows [98k, 98k+98) (6272 output rows). Per core:
  - bulk-gather seq[cols] HBM->SBUF with nc.gpsimd.dma_gather (int16 indices,
    so cols are split into lo (<32768) / hi (>=32768) halves, each gathered
    from a base-offset view of seq; pad slots gather row 0 with weight 0),
  - build selection matrix S[p, r] = vals * (rows_adj == r) on the vector
    engine (iota constant + is_equal + mult with broadcast APs),
  - PE matmul S^T @ G accumulated per 64-row window in PSUM,
  - flush PSUM -> SBUF staging (partition-major), one DMA to HBM at the end.
No cross-core communication; the host splits edges and concatenates outputs.
"""

import sys

if "/opt/trn_rl_repo" not in sys.path:
    sys.path.insert(0, "/opt/trn_rl_repo")

import numpy as np

N_NODES = 50000
N_EDGES = 1_250_000
D_FEAT = 64
W = 64            # rows per window
NW = 98           # windows per core
NCORE = 8
CHUNK_WINDOWS = 8
HALF = 32768      # int16 index limit

_compiled = {}


def _build_kernel(plan):
    from concourse import bass, bacc, mybir
    import concourse.tile as tile

    f32 = mybir.dt.float32
    i16 = mybir.dt.int16

    CTOT, CLO, CHI = plan["CTOT"], plan["CLO"], plan["CHI"]

    nc = bacc.Bacc("TRN2", target_bir_lowering=False, debug=False)
    seq_t = nc.dram_tensor("seq", [N_NODES, D_FEAT], f32, kind="ExternalInput")
    idxlo_t = nc.dram_tensor("idxlo", [128, CLO * 8], i16, kind="ExternalInput")
    idxhi_t = nc.dram_tensor("idxhi", [128, CHI * 8], i16, kind="ExternalInput")
    radj_t = nc.dram_tensor("radj", [128, CTOT], f32, kind="ExternalInput")
    vals_t = nc.dram_tensor("vals", [128, CTOT], f32, kind="ExternalInput")
    out_t = nc.dram_tensor("out", [64, NW * W], f32, kind="ExternalOutput")

    with tile.TileContext(nc) as tc:
        with (
            tc.tile_pool(name="const", bufs=1) as constp,
            tc.tile_pool(name="meta", bufs=3) as metap,
            tc.tile_pool(name="g", bufs=3) as gp,
            tc.tile_pool(name="s", bufs=2) as sp,
            tc.tile_pool(name="ps", bufs=2, space="PSUM") as psp,
            tc.tile_pool(name="st", bufs=1) as stp,
        ):
            iota_t = constp.tile([128, 8, 64], f32, name="iota")
            nc.gpsimd.iota(
                iota_t[:],
                pattern=[[0, 8], [1, 64]],
                base=0,
                channel_multiplier=0,
                allow_small_or_imprecise_dtypes=True,
            )
            stage = stp.tile([64, NW * W], f32, name="stage")

            for ci, ch in enumerate(plan["chunks"]):
                Cc = ch["Cc"]
                c0 = ch["c0"]
                w0, nw = ch["w0"], ch["nw"]

                radc = metap.tile([128, Cc], f32, tag="radj")
                valc = metap.tile([128, Cc], f32, tag="vals")
                nc.sync.dma_start(out=radc[:], in_=radj_t[:, c0 : c0 + Cc])
                nc.sync.dma_start(out=valc[:], in_=vals_t[:, c0 : c0 + Cc])

                G = {}
                for half, (idx_t, n_t, base_lo, base_hi, Ch) in (
                    (0, (idxlo_t, ch["n_lo_valid"], 0, HALF, ch["Clo"])),
                    (1, (idxhi_t, ch["n_hi_valid"], HALF, N_NODES, ch["Chi"])),
                ):
                    if Ch == 0:
                        continue
                    hc0 = ch["lo_c0"] if half == 0 else ch["hi_c0"]
                    idxc = metap.tile([128, Ch * 8], i16, tag=f"idx{half}")
                    nc.sync.dma_start(
                        out=idxc[:], in_=idx_t[:, hc0 * 8 : (hc0 + Ch) * 8]
                    )
                    Gt = gp.tile([128, Ch, 64], f32, tag=f"g{half}")
                    nc.gpsimd.dma_gather(
                        Gt[:],
                        seq_t[base_lo:base_hi, :],
                        idxc[:],
                        Ch * 128,
                        n_t,
                        D_FEAT,
                        single_packet=False,
                    )
                    G[half] = Gt

                S = sp.tile([128, Cc, 64], f32, tag="s")
                for g0 in range(0, Cc, 8):
                    gg = min(8, Cc - g0)
                    nc.vector.tensor_tensor(
                        out=S[:, g0 : g0 + gg, :],
                        in0=iota_t[:, :gg, :],
                        in1=radc[:, g0 : g0 + gg].to_broadcast([128, gg, 64]),
                        op=mybir.AluOpType.is_equal,
                    )
                    nc.vector.tensor_tensor(
                        out=S[:, g0 : g0 + gg, :],
                        in0=S[:, g0 : g0 + gg, :],
                        in1=valc[:, g0 : g0 + gg].to_broadcast([128, gg, 64]),
                        op=mybir.AluOpType.mult,
                    )

                ps = psp.tile([64, 512], f32, tag="ps")
                for c in range(Cc):
                    half, hpos, wi, first, last = ch["tiles"][c]
                    j = wi - w0
                    nc.tensor.matmul(
                        out=ps[:, j * 64 : (j + 1) * 64],
                        lhsT=S[:, c, :],
                        rhs=G[half][:, hpos, :],
                        start=first,
                        stop=last,
                    )

                nc.vector.tensor_copy(
                    out=stage[:, w0 * 64 : (w0 + nw) * 64], in_=ps[:, : nw * 64]
                )

            nc.sync.dma_start(out=out_t[:, :], in_=stage[:, :])

    nc.compile()
    return nc


def _preprocess(seq, vals, rows, cols):
    rows = np.asarray(rows)
    cols = np.asarray(cols)
    vals = np.asarray(vals)

    w_of_edge = rows // W
    counts = np.bincount(w_of_edge, minlength=NCORE * NW)
    starts = np.concatenate([[0], np.cumsum(counts)])

    # per (core, window): lo/hi counts
    n_lo = np.zeros((NCORE, NW), np.int64)
    n_hi = np.zeros((NCORE, NW), np.int64)
    lo_masks = {}
    for k in range(NCORE):
        for i in range(NW):
            g = k * NW + i
            s, e = int(starts[g]), int(starts[g + 1])
            m = cols[s:e] < HALF
            lo_masks[(k, i)] = m
            n_lo[k, i] = int(m.sum())
            n_hi[k, i] = int((~m).sum())

    NLlo = np.maximum(1, -(-n_lo.max(axis=0) // 128))   # [NW] tiles, >=1
    NLhi = -(-n_hi.max(axis=0) // 128)                  # [NW] tiles, may be 0
    CLO, CHI = int(NLlo.sum()), int(NLhi.sum())
    CTOT = CLO + CHI

    # global tile order: per window, lo tiles then hi tiles
    lo_slot0 = np.concatenate([[0], np.cumsum(NLlo)])   # within lo half
    hi_slot0 = np.concatenate([[0], np.cumsum(NLhi)])   # within hi half
    tile_start_of_window = np.concatenate([[0], np.cumsum(NLlo + NLhi)])

    # chunk definitions (shared across cores); tile schedule per chunk
    chunks = []
    for w0 in range(0, NW, CHUNK_WINDOWS):
        nw = min(CHUNK_WINDOWS, NW - w0)
        c0 = int(tile_start_of_window[w0])
        Cc = int(tile_start_of_window[w0 + nw] - c0)
        lo_c0, hi_c0 = int(lo_slot0[w0]), int(hi_slot0[w0])
        Clo = int(lo_slot0[w0 + nw] - lo_c0)
        Chi = int(hi_slot0[w0 + nw] - hi_c0)
        tiles = []
        for i in range(w0, w0 + nw):
            ntile = int(NLlo[i] + NLhi[i])
            for t in range(ntile):
                half = 0 if t < NLlo[i] else 1
                hpos = (
                    int(lo_slot0[i] - lo_c0 + t)
                    if half == 0
                    else int(hi_slot0[i] - hi_c0 + t - NLlo[i])
                )
                tiles.append((half, hpos, i, t == 0, t == ntile - 1))
        chunks.append(
            dict(
                c0=c0, Cc=Cc, w0=w0, nw=nw,
                lo_c0=lo_c0, hi_c0=hi_c0, Clo=Clo, Chi=Chi,
                tiles=tiles, n_lo_valid=0, n_hi_valid=0,
            )
        )

    # pack per-core data
    idx_lo = np.zeros((NCORE, CLO * 128), np.int16)
    idx_hi = np.zeros((NCORE, CHI * 128), np.int16)
    radj_pad = np.zeros((NCORE, CTOT * 128), np.float32)
    vals_pad = np.zeros((NCORE, CTOT * 128), np.float32)
    for k in range(NCORE):
        for i in range(NW):
            g = k * NW + i
            s, e = int(starts[g]), int(starts[g + 1])
            m = lo_masks[(k, i)]
            c = cols[s:e]
            r = (rows[s:e] - g * W).astype(np.float32)
            v = vals[s:e]
            nlo = int(n_lo[k, i])
            nhi = int(n_hi[k, i])
            ls = int(lo_slot0[i]) * 128
            hs = int(hi_slot0[i]) * 128
            idx_lo[k, ls : ls + nlo] = c[m].astype(np.int16)
            idx_hi[k, hs : hs + nhi] = (c[~m] - HALF).astype(np.int16)
            ts = int(tile_start_of_window[i]) * 128
            hts = ts + int(NLlo[i]) * 128
            radj_pad[k, ts : ts + nlo] = r[m]
            vals_pad[k, ts : ts + nlo] = v[m]
            radj_pad[k, hts : hts + nhi] = r[~m]
            vals_pad[k, hts : hts + nhi] = v[~m]

    # All pad slots hold index 0 (a valid row of the half view, selection
    # weight 0), so every slot is gathered and num_idxs_reg == num_idxs is
    # identical across cores -- required because the SPMD program is shared.
    for ch in chunks:
        ch["n_lo_valid"] = ch["Clo"] * 128
        ch["n_hi_valid"] = ch["Chi"] * 128

    def wrap16(a):
        # [L] -> [128, L/16] int16, lanes-of-16 wrapped then replicated x8
        t = a.reshape(-1, 16).T
        return np.ascontiguousarray(np.tile(t, (8, 1)))

    seq2d = np.ascontiguousarray(np.asarray(seq).reshape(N_NODES, D_FEAT))
    in_maps = []
    for k in range(NCORE):
        in_maps.append(
            {
                "seq": seq2d,
                "idxlo": wrap16(idx_lo[k]),
                "idxhi": wrap16(idx_hi[k]),
                "radj": np.ascontiguousarray(
                    radj_pad[k].reshape(CTOT, 128).T
                ),
                "vals": np.ascontiguousarray(
                    vals_pad[k].reshape(CTOT, 128).T
                ),
            }
        )

    plan = dict(CTOT=CTOT, CLO=CLO, CHI=CHI, chunks=chunks)
    return plan, in_maps, (n_lo, n_hi)


def kernel(seq, vals, rows, cols, _trace=False):
    from concourse.bass_utils import run_bass_kernel_spmd

    plan, in_maps, _ = _preprocess(seq, vals, rows, cols)

    key = (
        plan["CTOT"], plan["CLO"], plan["CHI"],
        tuple((ch["Cc"], ch["Clo"], ch["Chi"]) for ch in plan["chunks"]),
    )
    if key not in _compiled:
        _compiled[key] = _build_kernel(plan)
    nc = _compiled[key]

    res = run_bass_kernel_spmd(nc, in_maps, core_ids=list(range(NCORE)), trace=_trace)

    outs = []
    for k in range(NCORE):
        o = res.results[k]["out"]                        # [64, 6272]
        outs.append(o.reshape(64, NW, 64).transpose(1, 0, 2).reshape(NW * W, 64))
    full = np.concatenate(outs, axis=0)[:N_NODES]
    out = full[None].astype(np.float32)
    if _trace:
        return out, res
    return out



# revision 2
# speedup vs baseline: 1.0225x; 1.0225x over previous
"""Edge-parallel COO SpMM on 8 trn2 cores — bf16 pair-token gather variant.

out[r] = sum_{e: rows[e]==r} vals[e] * seq[0, cols[e], :]

rows sorted -> core k owns 98 contiguous 64-row windows (6272 rows). Per core:
  - tab[t] = [bf16(seq[2t]) | bf16(seq[2t+1])]  (256B tokens, 25000 rows), so
    gather indices are cols>>1 <= 24999 and fit int16 without a lo/hi split
    (the split cost the old kernel ~6% extra descriptor slots).
  - one dma_gather per 8-window chunk -> G [128, Cc, 128] bf16
  - DVE builds onehot = is_equal(iota, radj); S_even = onehot*veven,
    S_odd = onehot*vodd where veven/vodd = vals * (1 -/+ parity)/...
    (host folds the col parity into the two weight vectors)
  - PE per tile: psum += S_even^T @ G[:, :, 0:64] + S_odd^T @ G[:, :, 64:128]
    (bf16 matmuls; the wrong-parity half of each slot gets weight 0)
  - flush psum -> stage -> one DMA to HBM.
The Q7 SWDGE descriptor generation (~8ns/edge-slot) is the hard bottleneck;
everything else overlaps under it.
"""

import sys

if "/opt/trn_rl_repo" not in sys.path:
    sys.path.insert(0, "/opt/trn_rl_repo")

import numpy as np
import ml_dtypes

N_NODES = 50000
N_EDGES = 1_250_000
D_FEAT = 64
W = 64            # rows per window
NW = 98           # windows per core
NCORE = 8
CHUNK_WINDOWS = 8
NTOK = N_NODES // 2

_compiled = {}


def _build_kernel(plan):
    from concourse import bass, bacc, mybir
    import concourse.tile as tile

    f32 = mybir.dt.float32
    bf16 = mybir.dt.bfloat16
    i16 = mybir.dt.int16

    CTOT = plan["CTOT"]

    nc = bacc.Bacc("TRN2", target_bir_lowering=False, debug=False)
    tab_t = nc.dram_tensor("tab", [NTOK, 128], bf16, kind="ExternalInput")
    idx_t = nc.dram_tensor("idx", [128, CTOT * 8], i16, kind="ExternalInput")
    radj_t = nc.dram_tensor("radj", [128, CTOT], bf16, kind="ExternalInput")
    vev_t = nc.dram_tensor("vev", [128, CTOT], bf16, kind="ExternalInput")
    vod_t = nc.dram_tensor("vod", [128, CTOT], bf16, kind="ExternalInput")
    out_t = nc.dram_tensor("out", [64, NW * W], f32, kind="ExternalOutput")

    with tile.TileContext(nc) as tc:
        with (
            tc.tile_pool(name="const", bufs=1) as constp,
            tc.tile_pool(name="meta", bufs=3) as metap,
            tc.tile_pool(name="g", bufs=2) as gp,
            tc.tile_pool(name="s", bufs=2) as sp,
            tc.tile_pool(name="ps", bufs=2, space="PSUM") as psp,
            tc.tile_pool(name="st", bufs=1) as stp,
        ):
            iota_t = constp.tile([128, 64], bf16, name="iota")
            nc.gpsimd.iota(
                iota_t[:],
                pattern=[[1, 64]],
                base=0,
                channel_multiplier=0,
                allow_small_or_imprecise_dtypes=True,
            )
            stage = stp.tile([64, NW * W], f32, name="stage")

            for ch in plan["chunks"]:
                Cc = ch["Cc"]
                c0 = ch["c0"]
                w0, nw = ch["w0"], ch["nw"]

                radc = metap.tile([128, Cc], bf16, tag="radj")
                vec = metap.tile([128, Cc], bf16, tag="vev")
                voc = metap.tile([128, Cc], bf16, tag="vod")
                idxc = metap.tile([128, Cc * 8], i16, tag="idx")
                nc.sync.dma_start(out=idxc[:], in_=idx_t[:, c0 * 8 : (c0 + Cc) * 8])
                nc.sync.dma_start(out=radc[:], in_=radj_t[:, c0 : c0 + Cc])
                nc.sync.dma_start(out=vec[:], in_=vev_t[:, c0 : c0 + Cc])
                nc.sync.dma_start(out=voc[:], in_=vod_t[:, c0 : c0 + Cc])

                # two half-gathers per chunk: ~6.5k descriptors each, so two
                # can sit in the 1024-desc/engine SWDGE carveout and the next
                # desc-gen overlaps the previous drain
                G = gp.tile([128, Cc, 128], bf16, tag="g")
                Ch = (Cc + 1) // 2
                nc.gpsimd.dma_gather(
                    G[:, 0:Ch, :], tab_t[:, :], idxc[:, 0 : Ch * 8],
                    Ch * 128, Ch * 128, 128,
                    single_packet=False,
                )
                nc.gpsimd.dma_gather(
                    G[:, Ch:Cc, :], tab_t[:, :], idxc[:, Ch * 8 : Cc * 8],
                    (Cc - Ch) * 128, (Cc - Ch) * 128, 128,
                    single_packet=False,
                )

                Se = sp.tile([128, Cc, 64], bf16, tag="se")
                So = sp.tile([128, Cc, 64], bf16, tag="so")
                # one-hot into Se, then So = Se*vodd, Se = Se*veven (one big
                # op each; iota broadcast along the tile dim, weights along
                # the feature dim)
                nc.vector.tensor_tensor(
                    out=Se[:, :, :],
                    in0=iota_t[:, None, :].to_broadcast([128, Cc, 64]),
                    in1=radc[:, :].to_broadcast([128, Cc, 64]),
                    op=mybir.AluOpType.is_equal,
                )
                nc.vector.tensor_tensor(
                    out=So[:, :, :],
                    in0=Se[:, :, :],
                    in1=voc[:, :].to_broadcast([128, Cc, 64]),
                    op=mybir.AluOpType.mult,
                )
                nc.vector.tensor_tensor(
                    out=Se[:, :, :],
                    in0=Se[:, :, :],
                    in1=vec[:, :].to_broadcast([128, Cc, 64]),
                    op=mybir.AluOpType.mult,
                )

                ps = psp.tile([64, 512], f32, tag="ps")
                for c in range(Cc):
                    wi, first, last = ch["tiles"][c]
                    j = wi - w0
                    nc.tensor.matmul(
                        out=ps[:, j * 64 : (j + 1) * 64],
                        lhsT=Se[:, c, :],
                        rhs=G[:, c, 0:64],
                        start=first,
                        stop=False,
                    )
                    nc.tensor.matmul(
                        out=ps[:, j * 64 : (j + 1) * 64],
                        lhsT=So[:, c, :],
                        rhs=G[:, c, 64:128],
                        start=False,
                        stop=last,
                    )

                nc.scalar.activation(
                    out=stage[:, w0 * 64 : (w0 + nw) * 64],
                    in_=ps[:, : nw * 64],
                    func=mybir.ActivationFunctionType.Copy,
                )

            nc.sync.dma_start(out=out_t[:, :], in_=stage[:, :])

    nc.compile()
    return nc


def _preprocess(seq, vals, rows, cols):
    rows = np.asarray(rows)
    cols = np.asarray(cols)
    vals = np.asarray(vals)

    w_of_edge = rows // W
    counts = np.bincount(w_of_edge, minlength=NCORE * NW)
    starts = np.concatenate([[0], np.cumsum(counts)])
    cnt = counts.reshape(NCORE, NW)

    NT = np.maximum(1, -(-cnt.max(axis=0) // 128))     # [NW] tiles per window
    CTOT = int(NT.sum())
    tile_start = np.concatenate([[0], np.cumsum(NT)])

    chunks = []
    for w0 in range(0, NW, CHUNK_WINDOWS):
        nw = min(CHUNK_WINDOWS, NW - w0)
        c0 = int(tile_start[w0])
        Cc = int(tile_start[w0 + nw] - c0)
        tiles = []
        for i in range(w0, w0 + nw):
            nt = int(NT[i])
            for t in range(nt):
                tiles.append((i, t == 0, t == nt - 1))
        chunks.append(dict(c0=c0, Cc=Cc, w0=w0, nw=nw, tiles=tiles))

    # pack per-core slot data
    idx16 = np.zeros((NCORE, CTOT * 128), np.int16)
    radj = np.zeros((NCORE, CTOT * 128), np.float32)
    vev = np.zeros((NCORE, CTOT * 128), np.float32)
    vod = np.zeros((NCORE, CTOT * 128), np.float32)
    for k in range(NCORE):
        for i in range(NW):
            g = k * NW + i
            s, e = int(starts[g]), int(starts[g + 1])
            n = e - s
            ts = int(tile_start[i]) * 128
            c = cols[s:e]
            par = (c & 1).astype(np.float32)
            idx16[k, ts : ts + n] = (c >> 1).astype(np.int16)
            radj[k, ts : ts + n] = (rows[s:e] - g * W).astype(np.float32)
            v = vals[s:e]
            vev[k, ts : ts + n] = v * (1.0 - par)
            vod[k, ts : ts + n] = v * par

    def wrap16(a):
        t = a.reshape(-1, 16).T
        return np.ascontiguousarray(np.tile(t, (8, 1)))

    seqb = np.asarray(seq).reshape(N_NODES, D_FEAT).astype(ml_dtypes.bfloat16)
    tab = np.ascontiguousarray(seqb.reshape(NTOK, 128))

    bf = ml_dtypes.bfloat16
    in_maps = []
    for k in range(NCORE):
        in_maps.append(
            {
                "tab": tab,
                "idx": wrap16(idx16[k]),
                "radj": np.ascontiguousarray(radj[k].reshape(CTOT, 128).T).astype(bf),
                "vev": np.ascontiguousarray(vev[k].reshape(CTOT, 128).T).astype(bf),
                "vod": np.ascontiguousarray(vod[k].reshape(CTOT, 128).T).astype(bf),
            }
        )

    plan = dict(CTOT=CTOT, chunks=chunks)
    return plan, in_maps


def kernel(seq, vals, rows, cols, _trace=False):
    from concourse.bass_utils import run_bass_kernel_spmd

    plan, in_maps = _preprocess(seq, vals, rows, cols)

    key = (plan["CTOT"], tuple(ch["Cc"] for ch in plan["chunks"]))
    if key not in _compiled:
        _compiled[key] = _build_kernel(plan)
    nc = _compiled[key]

    res = run_bass_kernel_spmd(nc, in_maps, core_ids=list(range(NCORE)), trace=_trace)

    outs = []
    for k in range(NCORE):
        o = res.results[k]["out"]                        # [64, 6272]
        outs.append(o.reshape(64, NW, 64).transpose(1, 0, 2).reshape(NW * W, 64))
    full = np.concatenate(outs, axis=0)[:N_NODES]
    out = full[None].astype(np.float32)
    if _trace:
        return out, res
    return out


# revision 3
# speedup vs baseline: 1.0279x; 1.0053x over previous
"""Edge-parallel COO SpMM on 8 trn2 cores — bf16 pair-token gather variant.

out[r] = sum_{e: rows[e]==r} vals[e] * seq[0, cols[e], :]

rows sorted -> core k owns 98 contiguous 64-row windows (6272 rows). Per core:
  - tab[t] = [bf16(seq[2t]) | bf16(seq[2t+1])]  (256B tokens, 25000 rows), so
    gather indices are cols>>1 <= 24999 and fit int16 without a lo/hi split
    (the split cost the old kernel ~6% extra descriptor slots).
  - one dma_gather per 8-window chunk -> G [128, Cc, 128] bf16
  - DVE builds onehot = is_equal(iota, radj); S_even = onehot*veven,
    S_odd = onehot*vodd where veven/vodd = vals * (1 -/+ parity)/...
    (host folds the col parity into the two weight vectors)
  - PE per tile: psum += S_even^T @ G[:, :, 0:64] + S_odd^T @ G[:, :, 64:128]
    (bf16 matmuls; the wrong-parity half of each slot gets weight 0)
  - flush psum -> stage -> one DMA to HBM.
The Q7 SWDGE descriptor generation (~8ns/edge-slot) is the hard bottleneck;
everything else overlaps under it.
"""

import sys

if "/opt/trn_rl_repo" not in sys.path:
    sys.path.insert(0, "/opt/trn_rl_repo")

import numpy as np
import ml_dtypes

N_NODES = 50000
N_EDGES = 1_250_000
D_FEAT = 64
W = 128           # rows per window
NW = 49           # windows per core
NCORE = 8
CHUNK_WINDOWS = 4
NTOK = N_NODES // 2

_compiled = {}


def _build_kernel(plan):
    from concourse import bass, bacc, mybir
    import concourse.tile as tile

    f32 = mybir.dt.float32
    bf16 = mybir.dt.bfloat16
    i16 = mybir.dt.int16

    CTOT = plan["CTOT"]

    nc = bacc.Bacc("TRN2", target_bir_lowering=False, debug=False)
    tab_t = nc.dram_tensor("tab", [NTOK, 128], bf16, kind="ExternalInput")
    idx_t = nc.dram_tensor("idx", [128, CTOT * 8], i16, kind="ExternalInput")
    radj_t = nc.dram_tensor("radj", [128, CTOT], bf16, kind="ExternalInput")
    vev_t = nc.dram_tensor("vev", [128, CTOT], bf16, kind="ExternalInput")
    vod_t = nc.dram_tensor("vod", [128, CTOT], bf16, kind="ExternalInput")
    out_t = nc.dram_tensor("out", [128, NW * 64], f32, kind="ExternalOutput")

    with tile.TileContext(nc) as tc:
        with (
            tc.tile_pool(name="const", bufs=1) as constp,
            tc.tile_pool(name="meta", bufs=3) as metap,
            tc.tile_pool(name="g", bufs=2) as gp,
            tc.tile_pool(name="s", bufs=2) as sp,
            tc.tile_pool(name="ps", bufs=2, space="PSUM") as psp,
            tc.tile_pool(name="st", bufs=1) as stp,
        ):
            iota_t = constp.tile([128, 128], bf16, name="iota")
            nc.gpsimd.iota(
                iota_t[:],
                pattern=[[1, 128]],
                base=0,
                channel_multiplier=0,
                allow_small_or_imprecise_dtypes=True,
            )
            stage = stp.tile([128, NW * 64], f32, name="stage")

            for ch in plan["chunks"]:
                Cc = ch["Cc"]
                c0 = ch["c0"]
                w0, nw = ch["w0"], ch["nw"]

                radc = metap.tile([128, Cc], bf16, tag="radj")
                vec = metap.tile([128, Cc], bf16, tag="vev")
                voc = metap.tile([128, Cc], bf16, tag="vod")
                idxc = metap.tile([128, Cc * 8], i16, tag="idx")
                nc.sync.dma_start(out=idxc[:], in_=idx_t[:, c0 * 8 : (c0 + Cc) * 8])
                nc.sync.dma_start(out=radc[:], in_=radj_t[:, c0 : c0 + Cc])
                nc.sync.dma_start(out=vec[:], in_=vev_t[:, c0 : c0 + Cc])
                nc.sync.dma_start(out=voc[:], in_=vod_t[:, c0 : c0 + Cc])

                # two half-gathers per chunk: ~6.5k descriptors each, so two
                # can sit in the 1024-desc/engine SWDGE carveout and the next
                # desc-gen overlaps the previous drain
                G = gp.tile([128, Cc, 128], bf16, tag="g")
                Ch = (Cc + 1) // 2
                nc.gpsimd.dma_gather(
                    G[:, 0:Ch, :], tab_t[:, :], idxc[:, 0 : Ch * 8],
                    Ch * 128, Ch * 128, 128,
                    single_packet=False,
                )
                nc.gpsimd.dma_gather(
                    G[:, Ch:Cc, :], tab_t[:, :], idxc[:, Ch * 8 : Cc * 8],
                    (Cc - Ch) * 128, (Cc - Ch) * 128, 128,
                    single_packet=False,
                )

                ps = psp.tile([128, 256], f32, tag="ps")
                for h0, h1 in ((0, Ch), (Ch, Cc)):
                    hc = h1 - h0
                    Se = sp.tile([128, hc, 128], bf16, tag=f"se{h0 > 0}")
                    So = sp.tile([128, hc, 128], bf16, tag=f"so{h0 > 0}")
                    nc.vector.tensor_tensor(
                        out=Se[:, :, :],
                        in0=iota_t[:, None, :].to_broadcast([128, hc, 128]),
                        in1=radc[:, h0:h1].to_broadcast([128, hc, 128]),
                        op=mybir.AluOpType.is_equal,
                    )
                    nc.vector.tensor_tensor(
                        out=So[:, :, :],
                        in0=Se[:, :, :],
                        in1=voc[:, h0:h1].to_broadcast([128, hc, 128]),
                        op=mybir.AluOpType.mult,
                    )
                    nc.vector.tensor_tensor(
                        out=Se[:, :, :],
                        in0=Se[:, :, :],
                        in1=vec[:, h0:h1].to_broadcast([128, hc, 128]),
                        op=mybir.AluOpType.mult,
                    )
                    for c in range(h0, h1):
                        wi, first, last = ch["tiles"][c]
                        j = wi - w0
                        nc.tensor.matmul(
                            out=ps[:, j * 64 : (j + 1) * 64],
                            lhsT=Se[:, c - h0, :],
                            rhs=G[:, c, 0:64],
                            start=first,
                            stop=False,
                        )
                        nc.tensor.matmul(
                            out=ps[:, j * 64 : (j + 1) * 64],
                            lhsT=So[:, c - h0, :],
                            rhs=G[:, c, 64:128],
                            start=False,
                            stop=last,
                        )

                nc.scalar.activation(
                    out=stage[:, w0 * 64 : (w0 + nw) * 64],
                    in_=ps[:, : nw * 64],
                    func=mybir.ActivationFunctionType.Copy,
                )

            nc.sync.dma_start(out=out_t[:, :], in_=stage[:, :])

    nc.compile()
    return nc


def _preprocess(seq, vals, rows, cols):
    rows = np.asarray(rows)
    cols = np.asarray(cols)
    vals = np.asarray(vals)

    w_of_edge = rows // W
    counts = np.bincount(w_of_edge, minlength=NCORE * NW)
    starts = np.concatenate([[0], np.cumsum(counts)])
    cnt = counts.reshape(NCORE, NW)

    NT = np.maximum(1, -(-cnt.max(axis=0) // 128))     # [NW] tiles per window
    CTOT = int(NT.sum())
    tile_start = np.concatenate([[0], np.cumsum(NT)])

    chunks = []
    for w0 in range(0, NW, CHUNK_WINDOWS):
        nw = min(CHUNK_WINDOWS, NW - w0)
        c0 = int(tile_start[w0])
        Cc = int(tile_start[w0 + nw] - c0)
        tiles = []
        for i in range(w0, w0 + nw):
            nt = int(NT[i])
            for t in range(nt):
                tiles.append((i, t == 0, t == nt - 1))
        chunks.append(dict(c0=c0, Cc=Cc, w0=w0, nw=nw, tiles=tiles))

    # pack per-core slot data
    idx16 = np.zeros((NCORE, CTOT * 128), np.int16)
    radj = np.zeros((NCORE, CTOT * 128), np.float32)
    vev = np.zeros((NCORE, CTOT * 128), np.float32)
    vod = np.zeros((NCORE, CTOT * 128), np.float32)
    for k in range(NCORE):
        for i in range(NW):
            g = k * NW + i
            s, e = int(starts[g]), int(starts[g + 1])
            n = e - s
            ts = int(tile_start[i]) * 128
            c = cols[s:e]
            par = (c & 1).astype(np.float32)
            idx16[k, ts : ts + n] = (c >> 1).astype(np.int16)
            radj[k, ts : ts + n] = (rows[s:e] - g * W).astype(np.float32)
            v = vals[s:e]
            vev[k, ts : ts + n] = v * (1.0 - par)
            vod[k, ts : ts + n] = v * par

    def wrap16(a):
        t = a.reshape(-1, 16).T
        return np.ascontiguousarray(np.tile(t, (8, 1)))

    seqb = np.asarray(seq).reshape(N_NODES, D_FEAT).astype(ml_dtypes.bfloat16)
    tab = np.ascontiguousarray(seqb.reshape(NTOK, 128))

    bf = ml_dtypes.bfloat16
    in_maps = []
    for k in range(NCORE):
        in_maps.append(
            {
                "tab": tab,
                "idx": wrap16(idx16[k]),
                "radj": np.ascontiguousarray(radj[k].reshape(CTOT, 128).T).astype(bf),
                "vev": np.ascontiguousarray(vev[k].reshape(CTOT, 128).T).astype(bf),
                "vod": np.ascontiguousarray(vod[k].reshape(CTOT, 128).T).astype(bf),
            }
        )

    plan = dict(CTOT=CTOT, chunks=chunks)
    return plan, in_maps


def kernel(seq, vals, rows, cols, _trace=False):
    from concourse.bass_utils import run_bass_kernel_spmd

    plan, in_maps = _preprocess(seq, vals, rows, cols)

    key = (plan["CTOT"], tuple(ch["Cc"] for ch in plan["chunks"]))
    if key not in _compiled:
        _compiled[key] = _build_kernel(plan)
    nc = _compiled[key]

    res = run_bass_kernel_spmd(nc, in_maps, core_ids=list(range(NCORE)), trace=_trace)

    outs = []
    for k in range(NCORE):
        o = res.results[k]["out"]                        # [128, 3136]
        outs.append(o.reshape(128, NW, 64).transpose(1, 0, 2).reshape(NW * W, 64))
    full = np.concatenate(outs, axis=0)[:N_NODES]
    out = full[None].astype(np.float32)
    if _trace:
        return out, res
    return out
